# revision 50
# baseline (speedup 1.0000x reference)
"""Multi-head self-attention (B=2, T=2048, C=1024, H=16) on 8 trn2 cores.

Sharding: core c -> batch b = c//4, heads 4*(c%4) .. 4*(c%4)+3.
Each core: QKV projection for its 4 heads, causal attention in S^T layout
(keys on partitions), partial output projection over its heads' rows of Wo.
Host sums the 4 partials per batch element and adds bo.

Structure (v3):
- all operand streams in bf16 (q/k/pt/v/attn/wo/masks and the output
  partials); PSUM accumulation stays fp32. Kills the fp32r narrow-matmul
  penalty, halves DVE/DMA bytes.
- softmax division is per-slab: reciprocal straight off the PV denominator
  PSUM row, gpsimd partition-broadcast, multiply fused into the PSUM->SBUF
  copy. Division for slab s is emitted inside slab s+1.
- output projection interleaved into the attention loop, sharing the
  S-matmul PSUM ring; its DMA drains during attention.
- final slab: per-block division + output projection emitted inline as the
  block's last PV chunk lands, so the tail is one block deep (keeps the
  HAM clock at 8/8 through the end and overlaps the last output DMAs).
- warmup matmuls run on a memset tile (no DMA dependency), releasing the
  HAM clock throttle before the first real matmul.
"""
import sys

sys.path.insert(0, "/opt/trn_rl_repo")

import numpy as np

B, T, C, H = 2, 2048, 1024, 16
HD = C // H            # 64
NCORES = 8
HPC = H // (NCORES // B)   # heads per core = 4
QB = 128               # q block (columns of S^T)
KB = 128               # k chunk (partitions of S^T)
NJ = T // KB           # 16
NI = T // QB           # 16
SLAB = 512             # q columns processed per attention pass
NSLAB = T // SLAB      # 4
BPS = SLAB // QB       # q blocks per slab = 4
CI = C // 128          # 8 contraction chunks for projections
SCALE = HD ** -0.5

_cache = {}


def _build_plan(mask_bool: np.ndarray):
    """mask_bool: [T, T] (q, k). Returns per (j, i) block types and tiles.

    type 0 = all valid (no mask work), 1 = all masked (skip), 2 = mixed.
    Tiles are stored transposed to match S^T ([k_local, q_local])."""
    btype = np.zeros((NJ, NI), dtype=np.int32)
    tidx = np.full((NJ, NI), -1, dtype=np.int32)
    tiles = []
    tile_map = {}
    for j in range(NJ):
        for i in range(NI):
            sub = mask_bool[i * QB:(i + 1) * QB, j * KB:(j + 1) * KB]
            if sub.all():
                btype[j, i] = 0
            elif not sub.any():
                btype[j, i] = 1
            else:
                btype[j, i] = 2
                key = sub.tobytes()
                if key not in tile_map:
                    tile_map[key] = len(tiles)
                    tiles.append(sub.T.astype(np.float32))
                tidx[j, i] = tile_map[key]
    if not tiles:
        tiles.append(np.ones((KB, QB), dtype=np.float32))
    return btype, tidx, np.stack(tiles)


def _build_program(btype, tidx, n_tiles, apply_qk_bias, apply_v_bias):
    import concourse.bass as bass
    import concourse.tile as tile
    import concourse.mybir as mybir
    from concourse import bacc

    F32 = mybir.dt.float32
    AF = mybir.ActivationFunctionType
    MULT = mybir.AluOpType.mult

    nc = bacc.Bacc("TRN2", target_bir_lowering=False, debug=False)
    BF16 = mybir.dt.bfloat16
    xt_d = nc.dram_tensor("xt", [C, T], BF16, kind="ExternalInput").ap()
    wqk_d = nc.dram_tensor("wqk", [C, 4 * 128], BF16, kind="ExternalInput").ap()
    wv_d = nc.dram_tensor("wv", [C, HPC * HD], BF16, kind="ExternalInput").ap()
    wo_d = nc.dram_tensor("wo", [HPC * HD, C], BF16, kind="ExternalInput").ap()
    mask_d = nc.dram_tensor("masks", [n_tiles, KB, QB], BF16,
                            kind="ExternalInput").ap()
    bqk_d = nc.dram_tensor("bqk", [128, 4], F32, kind="ExternalInput").ap()
    bv_d = nc.dram_tensor("bv", [128, 2], F32, kind="ExternalInput").ap()
    out_d = nc.dram_tensor("out", [T, C], BF16, kind="ExternalOutput").ap()

    with tile.TileContext(nc) as tc:
        with tc.tile_pool(name="weights", bufs=1) as wpool, \
             tc.tile_pool(name="acts", bufs=1) as apool:
            # ---- resident SBUF tensors ----
            wo = wpool.tile([128, 2, C], BF16)
            masks = wpool.tile([128, n_tiles * QB], BF16)
            bqk = wpool.tile([128, 4], F32)
            bv = wpool.tile([128, 2], F32)
            # q tiles hold (q_hA | q_hB) on partitions 0-63 / 64-127.
            qp = [apool.tile([128, T], BF16, tag=f"qp{i}", name=f"qp{i}")
                  for i in range(2)]
            # k zero-padded per head so S matmuls present K=128.
            kz = [apool.tile([128, T], BF16, tag=f"kz{i}", name=f"kz{i}")
                  for i in range(4)]
            vaug = apool.tile([128, NJ, HPC * (HD + 1)], BF16)

            # ---- fused projection + attention + output projection ----
            # The QKV projection for token slab s is emitted immediately
            # before the attention slab s, so the PE sees one dense
            # instruction stream (keeps the HAM clock gate at 8/8).
            # All PSUM chains (projection, S, outproj) share the "sst" ring.
            with tc.tile_pool(name="xtp", bufs=1) as xtp, \
                 tc.tile_pool(name="attnp", bufs=1) as attnp:
              # projection inputs in bf16: same PE rate, half the DMA/SBUF
              xt = xtp.tile([128, CI, T], BF16)      # x^T, c_in chunked
              wqk = xtp.tile([128, CI, 512], BF16)
              wv = xtp.tile([128, CI, HPC * HD], BF16)
              warm = xtp.tile([128, 128], BF16)
              attn = [attnp.tile([128, T], BF16, tag=f"attn{p}",
                                 name=f"attn{p}") for p in range(2)]
              qt4 = T // 4
              # warmup tile is memset (no DMA dependency) so the HAM ramp
              # burns while the first inputs stream in
              nc.vector.tensor_copy(warm[:],
                                    nc.const_aps.tensor(0.0, (128, 128)))
              # DMA issue costs ~0.6-1us per instruction and queues are
              # per-engine: spread the first-needed transfers across the
              # sync/scalar/gpsimd queues so slab 0's inputs land ~3x
              # sooner, then stream the rest on sync.
              # The sync queue starts pumping earliest (~9us); scalar/gpsimd
              # queues lag a few us.  Everything slab 0 needs goes on sync,
              # in consumption order and co-sized chunks; later-slab bulk
              # rides the slower queues.
              wqk_s = wqk_d.rearrange("(ci p) f -> p ci f", p=128)
              xt_s = xt_d.rearrange("(ci p) f -> p ci f", p=128)
              nc.sync.dma_start(wqk[:, :, 0:128], wqk_s[:, :, 0:128])
              nc.sync.dma_start(xt[:, 0:4, 0:qt4], xt_s[:, 0:4, 0:qt4])
              nc.sync.dma_start(xt[:, 4:CI, 0:qt4], xt_s[:, 4:CI, 0:qt4])
              nc.sync.dma_start(wqk[:, :, 128:512], wqk_s[:, :, 128:512])
              for t in range(n_tiles):
                  nc.sync.dma_start(masks[:, t * QB:(t + 1) * QB], mask_d[t])
              if apply_qk_bias:
                  nc.sync.dma_start(bqk[:], bqk_d)
              if apply_v_bias:
                  nc.sync.dma_start(bv[:], bv_d)
              nc.sync.dma_start(wv[:], wv_d.rearrange("(ci p) f -> p ci f",
                                                      p=128))
              nc.sync.dma_start(xt[:, :, qt4:2 * qt4], xt_s[:, :, qt4:2 * qt4])
              nc.sync.dma_start(wo[:, 0, :], wo_d[0:128, :])
              nc.sync.dma_start(wo[:, 1, :], wo_d[128:256, :])
              nc.sync.dma_start(xt[:, :, 2 * qt4:3 * qt4],
                                xt_s[:, :, 2 * qt4:3 * qt4])
              nc.sync.dma_start(xt[:, :, 3 * qt4:T], xt_s[:, :, 3 * qt4:T])
              for p in range(2):
                  nc.vector.tensor_copy(
                      kz[2 * p][64:128, :],
                      nc.const_aps.tensor(0.0, (64, T)))
                  nc.vector.tensor_copy(
                      kz[2 * p + 1][0:64, :],
                      nc.const_aps.tensor(0.0, (64, T)))
              va = vaug[:].rearrange("p j (h d) -> p j h d", h=HPC)
              nc.vector.tensor_copy(
                  va[:, :, :, HD:HD + 1],
                  nc.const_aps.tensor(1.0, (128, NJ, HPC, 1)))
              # load the gpsimd PartitionBroadcast library while the PE is
              # still waiting on input DMAs (off the critical path)
              gwarm = apool.tile([128, 16], F32, tag="gwarm", name="gwarm")
              nc.gpsimd.partition_broadcast(
                  gwarm[:], nc.const_aps.tensor(1.0, (1, 16)))
              with tc.tile_pool(name="psattn", bufs=1, space="PSUM") as sp, \
                   tc.tile_pool(name="psout", bufs=1, space="PSUM") as op, \
                   tc.tile_pool(name="ptp", bufs=6) as ptp, \
                   tc.tile_pool(name="divp", bufs=3) as divp, \
                   tc.tile_pool(name="osb", bufs=3) as osb:
                  pending_div = []   # (pair, [out_ps hl0, hl1], s)
                  pending_proj = []  # (token-block index, emit epoch)
                  epoch = [0]        # increments at every (slab, pair) head

                  def divide_block(pair, tiles_hl, s, bsl, dtag):
                      # softmax divide for columns bsl of slab s: reciprocal
                      # straight off the PV denominator PSUM row, broadcast
                      # across partitions, multiply fused into the
                      # PSUM->SBUF copy.
                      ncols = bsl.stop - bsl.start
                      recs = []
                      for hl in range(2):
                          # custom-DVE recip can't read PSUM: stage the
                          # denominator row through SBUF first
                          sums = divp.tile([1, SLAB], F32, tag="sums",
                                           name="sums")
                          nc.vector.tensor_copy(sums[0:1, 0:ncols],
                                                tiles_hl[hl][HD:HD + 1, bsl])
                          rec1 = divp.tile([1, SLAB], F32, tag="rec1",
                                           name="rec1")
                          nc.vector.reciprocal_approx_fast(
                              rec1[0:1, 0:ncols], sums[0:1, 0:ncols])
                          rec128 = divp.tile([128, SLAB], F32,
                                             tag=f"rec128_{dtag}_{hl}",
                                             name="rec128", bufs=1)
                          nc.gpsimd.partition_broadcast(rec128[:, 0:ncols],
                                                        rec1[0:1, 0:ncols])
                          recs.append(rec128)
                      for hl in range(2):
                          dst = attn[pair][64 * hl:64 * hl + 64,
                                           s * SLAB + bsl.start:
                                           s * SLAB + bsl.stop]
                          nc.vector.tensor_tensor(
                              out=dst, in0=tiles_hl[hl][0:HD, bsl],
                              in1=recs[hl][64 * hl:64 * hl + 64, 0:ncols],
                              op=MULT)
                          if apply_v_bias:
                              nc.vector.tensor_scalar(
                                  out=dst, in0=dst,
                                  scalar1=bv[64 * hl:64 * hl + 64,
                                             pair:pair + 1],
                                  scalar2=None, op0=mybir.AluOpType.add)

                  def emit_division(pair, tiles_hl, s):
                      divide_block(pair, tiles_hl, s, slice(0, SLAB), "m")
                      if pair == 1:
                          pending_proj.extend(
                              (b, epoch[0])
                              for b in range(BPS * s, BPS * (s + 1)))

                  def emit_outproj(ts, tail=False):
                      # one 128-token block of the output projection; shares
                      # the S-matmul PSUM ring ("sst").
                      ps = sp.tile([128, 2, SLAB], F32, tag="sst",
                                   name="psop", bufs=3)
                      for h in range(2):
                          for pair in range(2):
                              nc.tensor.matmul(
                                  ps[:, h, :],
                                  attn[pair][:, ts * 128:(ts + 1) * 128],
                                  wo[:, pair, h * 512:(h + 1) * 512],
                                  start=(pair == 0), stop=(pair == 1))
                      ot = osb.tile([128, C], BF16, tag="ot", name="ot")
                      otv = ot[:].rearrange("p (h q) -> p h q", h=2)
                      if tail:
                          # Act is idle once the exp stream ends: tail copies
                          # there keep the DVE free for the division chains.
                          # Tail DMAs split across two queues (a single
                          # queue sustains only ~100GB/s at the tail).
                          nc.scalar.activation(otv, ps[:], AF.Identity,
                                               scale=1.0)
                          nc.scalar.dma_start(
                              out_d[ts * 128:ts * 128 + 64, :], ot[0:64, :])
                          nc.sync.dma_start(
                              out_d[ts * 128 + 64:(ts + 1) * 128, :],
                              ot[64:128, :])
                      else:
                          nc.vector.tensor_copy(otv, ps[:])
                          nc.sync.dma_start(
                              out_d[ts * 128:(ts + 1) * 128, :], ot[:])

                  def proj_chain_closures(s):
                      # QKV projection for token slab s (q/k for both pairs
                      # plus this slab's v blocks), through the sst ring.
                      # Returned as 8 closures so they can be spread between
                      # attention chunks (PE filler keeps the HAM clock warm).
                      sl = slice(s * 512, (s + 1) * 512)
                      chains = []
                      for co in range(4):
                          pair, is_k = co // 2, co % 2
                          def chain_qk(co=co, pair=pair, is_k=is_k):
                            ps = sp.tile([128, 2, SLAB], F32, tag="sst",
                                         name="psqk", bufs=3)
                            for ci in range(CI):
                                nc.tensor.matmul(
                                    ps[:, 0, :],
                                    wqk[:, ci, co * 128:(co + 1) * 128],
                                    xt[:, ci, sl],
                                    start=(ci == 0), stop=(ci == CI - 1))
                            if is_k:
                                dsts = [(kz[2 * pair][0:64, sl],
                                         ps[0:64, 0, :],
                                         bqk[0:64, co:co + 1]),
                                        (kz[2 * pair + 1][64:128, sl],
                                         ps[64:128, 0, :],
                                         bqk[64:128, co:co + 1])]
                            else:
                                dsts = [(qp[pair][:, sl], ps[:, 0, :],
                                         bqk[:, co:co + 1])]
                            for dst_ap, src_ap, b_ap in dsts:
                                if apply_qk_bias:
                                    nc.scalar.activation(dst_ap, src_ap,
                                                         AF.Identity,
                                                         bias=b_ap, scale=1.0)
                                else:
                                    nc.vector.tensor_copy(dst_ap, src_ap)
                          chains.append(chain_qk)
                      for tj in range(BPS * s, BPS * (s + 1)):
                          def chain_v(tj=tj):
                            ps = sp.tile([128, 2, SLAB], F32, tag="sst",
                                         name="psv", bufs=3)
                            for ci in range(CI):
                                nc.tensor.matmul(
                                    ps[:, 0, 0:HPC * HD],
                                    xt[:, ci, tj * 128:(tj + 1) * 128],
                                    wv[:, ci, :],
                                    start=(ci == 0), stop=(ci == CI - 1))
                            nc.vector.tensor_copy(
                                va[:, tj, :, 0:HD],
                                ps[:, 0, 0:HPC * HD].rearrange(
                                    "p (h d) -> p h d", h=HPC))
                          chains.append(chain_v)
                      return chains

                  # ~5us of dummy matmuls on the memset tile while the
                  # first projection inputs stream in: releases the HAM
                  # clock throttle before the first real matmul and keeps
                  # it released until the first chain's inputs land
                  for _w in range(3):
                      wps = sp.tile([128, 2, SLAB], F32, tag="sst",
                                    name="wps", bufs=3)
                      for r in range(16):
                          nc.tensor.matmul(wps[:, 0, 0:128], warm[:],
                                           warm[:], start=(r == 0),
                                           stop=(r == 15))
                  projq = []   # projection chains for the NEXT slab
                  for s in range(NSLAB):
                      for fn in projq:   # whatever wasn't spread: flush now
                          fn()
                      projq = (proj_chain_closures(s + 1)
                               if s + 1 < NSLAB else [])
                      if s == 0:
                          for fn in proj_chain_closures(0):
                              fn()
                      for pair in range(2):
                          epoch[0] += 1
                          final = (s == NSLAB - 1 and pair == 1)
                          q_t = qp[pair]
                          i_lo, i_hi = s * BPS, (s + 1) * BPS
                          chunks = []
                          for j in range(NJ):
                              live = [i for i in range(i_lo, i_hi)
                                      if btype[j, i] != 1]
                              if live:
                                  chunks.append((j, min(live), max(live)))
                          out_ps = [op.tile([HD + 1, SLAB], F32,
                                            tag=f"outps{_hl}",
                                            name=f"outps{_hl}", bufs=1)
                                    for _hl in range(2)]
                          written = np.zeros(BPS, dtype=bool)
                          # for the final slab: block b's columns are final
                          # after the PV of chunk j = last chunk touching it
                          last_j_for_block = {}
                          if final:
                              for (j, i0, i1) in chunks:
                                  for b in range(i0 - i_lo, i1 - i_lo + 1):
                                      last_j_for_block[b] = max(
                                          last_j_for_block.get(b, -1), j)

                          def emit_pv(j, i0, i1, pt, last):
                              r0 = i0 - i_lo
                              segs = []
                              c = r0 * QB
                              end = (i1 - i_lo + 1) * QB
                              while c < end:
                                  st = written[c // QB]
                                  cc = c + QB
                                  while cc < end and written[cc // QB] == st:
                                      cc += QB
                                  segs.append((c, cc, not st))
                                  c = cc
                              for hl in range(2):
                                  hh = 2 * pair + hl
                                  for (c0, c1, st_flag) in segs:
                                      nc.tensor.matmul(
                                          out_ps[hl][:, c0:c1],
                                          vaug[:, j, hh * (HD + 1):
                                               (hh + 1) * (HD + 1)],
                                          pt[:, hl, c0 - r0 * QB:c1 - r0 * QB],
                                          start=st_flag, stop=last,
                                          skip_group_check=True)
                              for rr in range(r0, i1 - i_lo + 1):
                                  written[rr] = True
                              # final slab: divide + project each token block
                              # as soon as its last PV chunk lands
                              if final:
                                  for b in range(BPS):
                                      if last_j_for_block.get(b) == j:
                                          divide_block(
                                              pair, out_ps, s,
                                              slice(b * QB, (b + 1) * QB),
                                              f"f{b}")
                                          emit_outproj(i_lo + b, tail=True)

                          pend_pv = []
                          for cn, (j, i0, i1) in enumerate(chunks):
                              n_cols = (i1 - i0 + 1) * QB
                              # S^T for both heads into the two banks of one
                              # psum tile; one exp covers both
                              sps = sp.tile([128, 2, SLAB], F32,
                                            tag="sst", name="sst", bufs=3)
                              for hl in range(2):
                                  nc.tensor.matmul(
                                      sps[:, hl, 0:n_cols],
                                      kz[2 * pair + hl][:,
                                                        j * KB:(j + 1) * KB],
                                      q_t[:, i0 * QB:i0 * QB + n_cols],
                                      start=True, stop=True)
                              pt = ptp.tile([128, 2, SLAB], BF16, tag="pt",
                                            name="pt")
                              nc.scalar.activation(pt[:, :, 0:n_cols],
                                                   sps[:, :, 0:n_cols],
                                                   AF.Exp, scale=SCALE)
                              # masks run on DVE so gpsimd only ever runs
                              # PartitionBroadcast (no Q7 library reloads)
                              for i in range(i0, i1 + 1):
                                  rel = (i - i0) * QB
                                  if btype[j, i] == 2:
                                      ti = tidx[j, i]
                                      m2 = masks[:, ti * QB:(ti + 1) * QB] \
                                          .unsqueeze(1).broadcast_to(
                                              [128, 2, QB])
                                      nc.vector.tensor_tensor(
                                          out=pt[:, :, rel:rel + QB],
                                          in0=pt[:, :, rel:rel + QB],
                                          in1=m2, op=MULT)
                                  elif btype[j, i] == 1:
                                      nc.vector.tensor_copy(
                                          pt[:, :, rel:rel + QB],
                                          nc.const_aps.tensor(
                                              0.0, (128, 2, QB)))
                              # previous slab's division goes out right after
                              # this slab's first chunk is in flight
                              if cn == 0 and pending_div:
                                  emit_division(*pending_div.pop(0))
                              # filler insertions go in pairs so the shared
                              # "sst" psum ring keeps its even double-buffer
                              # phase for the S-matmul pipeline; projection
                              # chains have priority over outproj blocks.
                              # At the pair head (cn 0/1) only blocks whose
                              # division is >= a full pair old may fill (a
                              # younger read would block the in-order PE
                              # FIFO on the division chain); the cn>=2 rule
                              # holds 2 such blocks back for the next head.
                              if cn < 2:
                                  if (pending_proj
                                          and pending_proj[0][1]
                                          <= epoch[0] - 1):
                                      emit_outproj(pending_proj.pop(0)[0])
                              elif cn % 2 == 0:
                                  for _ in range(2):
                                      if projq:
                                          projq.pop(0)()
                                      elif (len(pending_proj) > 2
                                            or (pending_proj
                                                and s == NSLAB - 1
                                                and pair == 1)):
                                          emit_outproj(
                                              pending_proj.pop(0)[0])
                              # PV lags two chunks behind S/exp so the PE
                              # never sits behind a recent exp, and the
                              # slab-head division gets two chunks of slack.
                              # In the final pair the lag tapers off so the
                              # per-block divisions start under the S stream
                              # instead of all after it.
                              pend_pv.append((j, i0, i1, pt))
                              # In the final pair the lag tapers off so the
                              # per-block divisions start under the S stream
                              # instead of all after it.
                              if final:
                                  lag = min(3, len(chunks) - 1 - cn)
                              else:
                                  lag = 3
                              while len(pend_pv) > lag:
                                  left = len(chunks) - 1 - cn + len(pend_pv)
                                  emit_pv(*pend_pv.pop(0), last=(left == 1))
                          for k, pv in enumerate(pend_pv):
                              emit_pv(*pv, last=(k == len(pend_pv) - 1))
                          pend_pv = []
                          if not final:
                              pending_div.append((pair, out_ps, s))
                  assert not pending_div
                  while pending_proj:
                      emit_outproj(pending_proj.pop(0)[0], tail=True)

    nc.compile()
    return nc


def _get_program(mask_bool, apply_qk_bias, apply_v_bias):
    key = (mask_bool.tobytes(), apply_qk_bias, apply_v_bias)
    if key not in _cache:
        btype, tidx, tiles = _build_plan(mask_bool)
        nc = _build_program(btype, tidx, len(tiles), apply_qk_bias,
                            apply_v_bias)
        _cache[key] = (nc, tiles)
    return _cache[key]


def kernel(x, attention_mask, Wqkv, bqkv, Wo, bo, _trace=False):
    import ml_dtypes
    from concourse.bass_utils import run_bass_kernel_spmd

    bf16 = ml_dtypes.bfloat16
    x = np.asarray(x, dtype=np.float32)
    mask_bool = np.asarray(attention_mask)[0, 0] != 0
    Wqkv = np.asarray(Wqkv, dtype=np.float32)
    bqkv = np.asarray(bqkv, dtype=np.float32)
    Wo = np.asarray(Wo, dtype=np.float32)
    bo = np.asarray(bo, dtype=np.float32)

    apply_qk_bias = bool(np.any(bqkv[:2 * C]))
    apply_v_bias = bool(np.any(bqkv[2 * C:]))
    nc, tiles = _get_program(mask_bool, apply_qk_bias, apply_v_bias)

    xts = [np.ascontiguousarray(x[b].T).astype(bf16) for b in range(B)]
    in_maps = []
    for c in range(NCORES):
        b, g = divmod(c, NCORES // B)
        hs = [HPC * g + i for i in range(HPC)]
        # wqk column chunks: [q_h0|q_h1, k_h0|k_h1, q_h2|q_h3, k_h2|k_h3]
        cols, bias_cols = [], []
        for pair in range(2):
            ha, hb = hs[2 * pair], hs[2 * pair + 1]
            for base in (0, C):  # q then k offset in Wqkv columns
                cols.append(Wqkv[:, base + ha * HD:base + (ha + 1) * HD])
                cols.append(Wqkv[:, base + hb * HD:base + (hb + 1) * HD])
                bias_cols.append(np.concatenate([
                    bqkv[base + ha * HD:base + (ha + 1) * HD],
                    bqkv[base + hb * HD:base + (hb + 1) * HD]]))
        wqk_c = np.concatenate(cols, axis=1).astype(bf16)
        bqk_c = np.stack(bias_cols, axis=1).astype(np.float32)
        wv_c = np.concatenate(
            [Wqkv[:, 2 * C + h * HD:2 * C + (h + 1) * HD] for h in hs],
            axis=1).astype(bf16)
        wo_c = np.concatenate(
            [Wo[h * HD:(h + 1) * HD, :] for h in hs], axis=0).astype(bf16)
        bv_c = np.zeros((128, 2), dtype=np.float32)
        for pair in range(2):
            ha, hb = hs[2 * pair], hs[2 * pair + 1]
            bv_c[0:HD, pair] = bqkv[2 * C + ha * HD:2 * C + (ha + 1) * HD]
            bv_c[HD:128, pair] = bqkv[2 * C + hb * HD:2 * C + (hb + 1) * HD]
        in_maps.append({
            "xt": xts[b], "wqk": wqk_c, "wv": wv_c, "wo": wo_c,
            "masks": tiles.astype(bf16),
            "bqk": bqk_c, "bv": bv_c,
        })

    kwargs = {}
    if _trace:
        kwargs = dict(trace=True, trace_cores=[0])
    res = run_bass_kernel_spmd(nc, in_maps, core_ids=list(range(NCORES)),
                               **kwargs)
    out = np.empty((B, T, C), dtype=np.float32)
    gpb = NCORES // B
    for b in range(B):
        acc = res.results[b * gpb]["out"].astype(np.float32)
        for g in range(1, gpb):
            acc = acc + res.results[b * gpb + g]["out"].astype(np.float32)
        out[b] = acc + bo
    if _trace:
        kernel._last_results = res
    return out
